# revision 6
# baseline (speedup 1.0000x reference)
"""Trainium2 Bass kernel for nn_DenseAttentionLayer (gnn_message_passing) — v3.

Math (reference):
    in_fts = context @ W_common.T            # (N, HID)
    left   = in_fts @ w_left + b_left        # (N,)
    right  = in_fts @ w_right + b_right      # (N,)
    logits = leaky_relu(left[:,None] + right[None,:], 0.2)
    logits = where(adj <= 0, -inf, logits)
    coefs  = softmax(logits, axis=-1)
    out    = relu(coefs @ relation)          # (N, REL_DIM)

Design:
  * TRANSPOSED elementwise layout: j on partitions, i on the free dim.
    The host uploads adj^T per core, so zm is produced directly in the
    lhsT orientation the P@V matmul needs -> no PE transposes, no
    PSUM->SBUF copies, no 4MB right-broadcast.
  * Two per-tile formulations, mixed at a g-fraction to balance ACT/DVE:
    - ACT form: zm = adj * Exp(Prelu(left_bcast + r_j_bias, alpha=0.2))
      — Prelu honors alpha on this HW and shares an act-table set with
      Exp, so exp(leaky(l+r)) is two fused-bias ACT ops; mask on DVE.
    - DVE form (rank-1 factorization): exp(x), x = l_i + r_j factorizes
      as e^l * e^r; with A=e^l, B=e^r, C=e^{0.2l}, D=e^{0.2r},
        zm_ij = adj_ij * max(A_i*B_j, C_i*D_j)
      = two 4x-mode tensor_scalar products + tensor_max + mask (2x).
  * DOT-PRODUCTS ON PE: context is uploaded transposed (ctxT [IN, N]),
    so right_j = ctx_j . v_right becomes 4 accumulating [128,128]x[128,1]
    matmuls per j-tile with the contraction on partitions — freeing the
    DVE/ACT engines (TensorTensor/TensorScalar are DVE-only on this
    toolchain; GPSIMD cannot run them).
  * PSUM packing: per-bank tiles. 4 banks hold 8 packed 256-wide output
    accumulators (two i-blocks per [128,512] bank tile), 1 bank holds
    the 8 softmax denominators (8 width-1 matmuls per j-tile against a
    ones column), 3 banks rotate for the PE dot-product chunks.

Sharding: row-shard the N x N logits; core c owns output rows
[c*R, (c+1)*R), R = N/8.  All params + full context replicated.
"""

import os
import sys

for _p in ("/opt/trn_rl_repo",):
    if _p not in sys.path and os.path.isdir(_p):
        sys.path.insert(0, _p)

from contextlib import ExitStack

import ml_dtypes
import numpy as np

# ---------------------------------------------------------------- constants
N = 8192  # num relations (columns j)
IN = 512  # context feature dim
D = 256  # relation dim (output dim)
NCORES = 8
P = 128
R = N // NCORES  # rows per core (i range)
KT = IN // P  # IN tiles (4)

_CACHE = {}


def _spread(frac, n):
    """n booleans with ~frac True, evenly spread."""
    out = []
    acc = 0.0
    for _ in range(n):
        acc += frac
        if acc >= 1.0 - 1e-9:
            acc -= 1.0
            out.append(True)
        else:
            out.append(False)
    return out


# ------------------------------------------------------------------ builder
def build_program(cfg):
    import concourse.bass as bass
    import concourse.tile as tile
    from concourse import bacc, mybir

    f32 = mybir.dt.float32
    bf16 = mybir.dt.bfloat16

    reps = cfg.get("reps", 1)
    g_act = cfg.get("g_act", 0.54)  # fraction of tiles on ACT (Prelu+Exp)
    lookahead = cfg.get("lookahead", 12)  # dot j-tiles emitted ahead
    njt = N // P  # 64 j-tiles
    NCH = 8  # j-tiles per dot chunk
    nch = njt // NCH  # 8 chunks
    ni = R // P  # 8 i-blocks
    NREL = 8  # rel load chunks

    Exp = mybir.ActivationFunctionType.Exp
    Relu = mybir.ActivationFunctionType.Relu
    Copy = mybir.ActivationFunctionType.Copy
    Prelu = mybir.ActivationFunctionType.Prelu
    Alu = mybir.AluOpType

    nc = bacc.Bacc("TRN2", target_bir_lowering=False, debug=False)

    adjt = nc.dram_tensor("adjt", [N, R], bf16, kind="ExternalInput")
    ctxt = nc.dram_tensor("ctxt", [IN, N], bf16, kind="ExternalInput")
    ctxot = nc.dram_tensor("ctxot", [IN, R], bf16, kind="ExternalInput")
    rel_in = nc.dram_tensor("rel_in", [N, D], bf16, kind="ExternalInput")
    vl_in = nc.dram_tensor("vl_in", [IN], bf16, kind="ExternalInput")
    vr_in = nc.dram_tensor("vr_in", [IN], bf16, kind="ExternalInput")
    # bias2 = [b_l + b_r, 0.2*(b_l + b_r)]
    bias2 = nc.dram_tensor("bias2", [2], f32, kind="ExternalInput")
    out = nc.dram_tensor("out", [R, D], f32, kind="ExternalOutput")
    l_scr = nc.dram_tensor("l_scr", [R], f32)
    a_scr = nc.dram_tensor("a_scr", [R], bf16)
    c_scr = nc.dram_tensor("c_scr", [R], bf16)

    t_sched = _spread(g_act, njt)  # True -> ACT-form tile (Prelu+Exp)

    with tile.TileContext(nc) as tc, ExitStack() as ctx:
        singles = ctx.enter_context(tc.tile_pool(name="singles", bufs=1))
        ctx_pool = ctx.enter_context(tc.tile_pool(name="ctxp", bufs=cfg.get("ctx_bufs", 5)))
        adj_pool = ctx.enter_context(tc.tile_pool(name="adjp", bufs=cfg.get("adj_bufs", 5)))
        rc_pool = ctx.enter_context(tc.tile_pool(name="rcp", bufs=cfg.get("rc_bufs", 7)))
        t_pool = ctx.enter_context(tc.tile_pool(name="tp", bufs=cfg.get("t_bufs", 6)))
        lk_pool = ctx.enter_context(tc.tile_pool(name="lkp", bufs=cfg.get("lk_bufs", 4)))
        m_pool_t = ctx.enter_context(tc.tile_pool(name="mp", bufs=cfg.get("m_bufs", 6)))
        zm_pool = ctx.enter_context(tc.tile_pool(name="zmp", bufs=cfg.get("zm_bufs", 8)))
        out_pool = ctx.enter_context(tc.tile_pool(name="outp", bufs=4))
        sm_pool = ctx.enter_context(tc.tile_pool(name="smp", bufs=4))
        acc_psum = ctx.enter_context(
            tc.tile_pool(name="accps", bufs=1, space="PSUM")
        )
        dot_psum = ctx.enter_context(
            tc.tile_pool(name="dotps", bufs=3, space="PSUM")
        )

        def _emit_body():
            # ---------------- singles / params ----------------
            # v_left / v_right in column layout [128, KT] (per-IN-tile cols)
            vlT = singles.tile([P, KT], bf16)
            nc.sync.dma_start(
                out=vlT, in_=bass.AP(tensor=vl_in, offset=0, ap=[[1, P], [P, KT]])
            )
            vrT = singles.tile([P, KT], bf16)
            nc.sync.dma_start(
                out=vrT, in_=bass.AP(tensor=vr_in, offset=0, ap=[[1, P], [P, KT]])
            )
            b2 = singles.tile([P, 2], f32)
            nc.sync.dma_start(
                out=b2, in_=bass.AP(tensor=bias2, offset=0, ap=[[0, P], [1, 2]])
            )
            ones_col = singles.tile([P, 1], bf16)
            nc.vector.memset(ones_col[:], 1.0)
            # warm the ACT function table at t~0 (the implicit
            # LoadActFuncSet otherwise lands mid-startup-chain, adding
            # ~1.3us to the first Prelu's latency)
            warm = singles.tile([P, 1], bf16)
            nc.scalar.activation(warm, ones_col, Exp, bias=0.0, scale=1.0)
            zeros_sb = singles.tile([P, 2 * D], bf16)
            nc.vector.memset(zeros_sb[:], 0.0)
            zid = singles.tile([P, P], bf16)
            nc.vector.memset(zid[:], 0.0)

            # relation tiles [P, njt, D], loaded in NREL chunks
            rel_sb = singles.tile([P, njt, D], bf16)
            relw = njt // NREL

            def emit_rel_chunk(rq):
                nc.sync.dma_start(
                    out=rel_sb[:, rq * relw : (rq + 1) * relw, :],
                    in_=bass.AP(tensor=rel_in, offset=rq * relw * P * D,
                                ap=[[D, P], [P * D, relw], [1, D]]),
                )

            # ---------------- right-dot machinery (PE) ----------------
            # ctx chunk q covers j-tiles [q*NCH, (q+1)*NCH)
            ctx_tiles = {}
            r_chunks = {}

            def emit_ctx_chunk(q):
                ct = ctx_pool.tile([P, KT, NCH * P], bf16, tag="ctx", name="ct")
                nc.sync.dma_start(
                    out=ct,
                    in_=bass.AP(
                        tensor=ctxt,
                        offset=q * NCH * P,
                        ap=[[N, P], [P * N, KT], [1, NCH * P]],
                    ),
                )
                ctx_tiles[q] = ct
                r_ps = dot_psum.tile([P, NCH], f32, tag="rdot", name="r_ps")
                r_chunks[q] = [r_ps, None, None, None]

            def emit_dot_tile(j):
                q, t = j // NCH, j % NCH
                if t == 0 and q not in ctx_tiles:
                    emit_ctx_chunk(q)
                r_ps = r_chunks[q][0]
                ct = ctx_tiles[q]
                for kt in range(KT):
                    nc.tensor.matmul(
                        r_ps[:, t : t + 1],
                        lhsT=ct[:, kt, t * P : (t + 1) * P],
                        rhs=vrT[:, kt : kt + 1],
                        start=(kt == 0),
                        stop=(kt == KT - 1),
                    )
                if t == NCH - 1:
                    # chunk complete: pull to SBUF + derive exp factors (ACT)
                    r_col = rc_pool.tile([P, NCH], f32, tag="rcol", name="r_col")
                    nc.scalar.activation(r_col, r_ps, Copy, bias=0.0, scale=1.0)
                    B_col = rc_pool.tile([P, NCH], f32, tag="bcol", name="B_col")
                    nc.scalar.activation(B_col, r_ps, Exp, bias=0.0, scale=1.0)
                    D_col = rc_pool.tile([P, NCH], f32, tag="dcol", name="D_col")
                    nc.scalar.activation(D_col, r_ps, Exp, bias=0.0, scale=0.2)
                    r_chunks[q][1:] = [r_col, B_col, D_col]

            def emit_adjt(jt):
                at = adj_pool.tile([P, 4, R], bf16, tag="adj", name="at")
                nc.sync.dma_start(
                    out=at,
                    in_=bass.AP(
                        tensor=adjt,
                        offset=jt * P * R,
                        ap=[[R, P], [P * R, 4], [1, R]],
                    ),
                )
                return at

            # ---- prefetch input streams (pure DMAs, nothing waits) ----
            own_ctx = singles.tile([P, KT, R], bf16)
            nc.sync.dma_start(
                out=own_ctx,
                in_=bass.AP(tensor=ctxot, offset=0,
                            ap=[[R, P], [P * R, KT], [1, R]]),
            )
            emit_ctx_chunk(0)
            adjt_tiles = {0: emit_adjt(0)}
            emit_rel_chunk(0)
            emit_ctx_chunk(1)
            adjt_tiles[1] = emit_adjt(4)
            emit_rel_chunk(1)

            # ---------------- left factors (own rows, PE dots) ----------
            l_ps = dot_psum.tile([P, ni], f32, tag="rdot", name="l_ps")
            for t in range(ni):
                for kt in range(KT):
                    nc.tensor.matmul(
                        l_ps[:, t : t + 1],
                        lhsT=own_ctx[:, kt, t * P : (t + 1) * P],
                        rhs=vlT[:, kt : kt + 1],
                        start=(kt == 0),
                        stop=(kt == KT - 1),
                    )
            # left = dot + b_left + b_right (both biases folded on the i side)
            left_col = singles.tile([P, ni], f32)
            nc.vector.tensor_scalar_add(left_col, l_ps, b2[:, 0:1])
            # tiny per-column exps, then bounce all three i-indexed vectors
            # to DRAM and broadcast back along partitions (keeps the big ACT
            # engine out of the startup critical path)
            A_col = singles.tile([P, ni], bf16)
            nc.scalar.activation(A_col, left_col, Exp, bias=0.0, scale=1.0)
            C_col = singles.tile([P, ni], bf16)
            nc.scalar.activation(C_col, left_col, Exp, bias=0.0, scale=0.2)
            nc.sync.dma_start(
                out=bass.AP(tensor=l_scr, offset=0, ap=[[1, P], [P, ni]]),
                in_=left_col[:, 0:ni],
            )
            nc.sync.dma_start(
                out=bass.AP(tensor=a_scr, offset=0, ap=[[1, P], [P, ni]]),
                in_=A_col[:, 0:ni],
            )
            nc.sync.dma_start(
                out=bass.AP(tensor=c_scr, offset=0, ap=[[1, P], [P, ni]]),
                in_=C_col[:, 0:ni],
            )
            left_bcast = singles.tile([P, R], f32)
            nc.sync.dma_start(
                out=left_bcast,
                in_=bass.AP(tensor=l_scr, offset=0, ap=[[0, P], [1, R]]),
            )
            A_bcast = singles.tile([P, R], bf16)
            nc.sync.dma_start(
                out=A_bcast,
                in_=bass.AP(tensor=a_scr, offset=0, ap=[[0, P], [1, R]]),
            )
            C_bcast = singles.tile([P, R], bf16)
            nc.sync.dma_start(
                out=C_bcast,
                in_=bass.AP(tensor=c_scr, offset=0, ap=[[0, P], [1, R]]),
            )

            for j in range(min(lookahead, njt)):
                emit_dot_tile(j)

            # ---------------- psum accumulators (packed) ----------------
            # two 256-wide i-block accumulators per [128,512] bank tile
            accab = []
            for ph in range(ni // 2):
                t_ = acc_psum.tile([P, 2 * D], f32, tag=f"accab{ph}", name=f"accab{ph}")
                accab.append(t_)
            denoms = acc_psum.tile([P, ni], f32, tag="denoms", name="denoms")
            # pre-zero the packed banks with single whole-bank zero-matmuls;
            # the per-j-tile matmuls then accumulate (start=False) only.
            # (a start=True write into a bank corrupts other in-flight
            # accumulation chains packed in the same bank)
            for ph in range(ni // 2):
                nc.tensor.matmul(
                    accab[ph][:], lhsT=zid[:], rhs=zeros_sb[:],
                    start=True, stop=True,
                )
            nc.tensor.matmul(
                denoms[:], lhsT=zid[:], rhs=zeros_sb[:, 0:ni],
                start=True, stop=True,
            )

            def acc_region(ib):
                return accab[ib // 2][:, (ib % 2) * D : (ib % 2 + 1) * D]

            # ---------------- main loop over j-tiles ----------------
            for jt in range(njt):
                q, tq = jt // NCH, jt % NCH
                if jt + lookahead < njt:
                    emit_dot_tile(jt + lookahead)
                if jt % 4 == 0 and jt + 8 < njt:
                    adjt_tiles[jt // 4 + 2] = emit_adjt(jt + 8)
                if jt % 4 == 2 and 2 + jt // 4 < NREL:
                    emit_rel_chunk(2 + jt // 4)
                adjt_tile = adjt_tiles[jt // 4]
                _, r_col, B_col, D_col = r_chunks[q]

                if t_sched[jt]:
                    # ACT form: exp(leaky(l_i + r_j)) in two fused ACT ops —
                    # Prelu honors alpha on this hardware (unlike Lrelu)
                    lk = lk_pool.tile([P, R], f32, tag="lk", name="lk")
                    nc.scalar.activation(
                        lk, left_bcast, Prelu,
                        bias=r_col[:, tq : tq + 1], scale=1.0, alpha=0.2,
                    )
                    mt = m_pool_t.tile([P, R], bf16, tag="m", name="mt")
                    nc.scalar.activation(mt, lk, Exp, bias=0.0, scale=1.0)
                else:
                    # DVE form: max(A_i*B_j, C_i*D_j) via 4x tensor_scalar
                    ts = []
                    for k in range(2):
                        tk = t_pool.tile([P, R], bf16, tag=f"t{k}", name="tk")
                        src_b = A_bcast if k == 0 else C_bcast
                        sc = B_col if k == 0 else D_col
                        nc.vector.tensor_scalar(
                            tk, src_b, sc[:, tq : tq + 1], None, Alu.mult
                        )
                        ts.append(tk)
                    mt = m_pool_t.tile([P, R], bf16, tag="m", name="mt")
                    nc.vector.tensor_max(mt, ts[0], ts[1])

                zm = zm_pool.tile([P, R], bf16, tag="zm", name="zm")
                nc.vector.tensor_tensor(
                    zm, mt, adjt_tile[:, jt % 4, :], op=Alu.mult
                )

                for ib in range(ni):
                    lhsT = zm[:, ib * P : (ib + 1) * P]
                    nc.tensor.matmul(
                        acc_region(ib),
                        lhsT=lhsT,
                        rhs=rel_sb[:, jt, :],
                        start=False,
                        stop=(jt == njt - 1),
                    )
                    nc.tensor.matmul(
                        denoms[:, ib : ib + 1],
                        lhsT=lhsT,
                        rhs=ones_col[:],
                        start=False,
                        stop=(jt == njt - 1),
                    )

            # ---------------- finalize ----------------
            for ib in range(ni):
                recip = sm_pool.tile([P, 1], f32, tag="recip", name="recip")
                nc.vector.reciprocal(recip, denoms[:, ib : ib + 1])
                ob = out_pool.tile([P, D], f32, tag="ob", name="ob")
                nc.scalar.activation(
                    ob, acc_region(ib), Relu, bias=0.0, scale=recip[:, 0:1]
                )
                nc.sync.dma_start(out=out[ib * P : (ib + 1) * P, :], in_=ob)

        if reps > 1:
            with tc.For_i(0, reps, 1):
                _emit_body()
        else:
            _emit_body()

    nc.compile()
    return nc


_BASE_CFG = dict(g_act=0.54)


def _get_program(cfg_key):
    if cfg_key not in _CACHE:
        _CACHE[cfg_key] = build_program(dict(_BASE_CFG))
    return _CACHE[cfg_key]


def prepare_in_maps(relation, context, adj_tensor, W_common, w_left, b_left,
                    w_right, b_right):
    bf = ml_dtypes.bfloat16
    relation = np.asarray(relation, dtype=np.float32)
    context = np.asarray(context, dtype=np.float32)
    adj_tensor = np.asarray(adj_tensor, dtype=np.float32)
    W_common = np.asarray(W_common, dtype=np.float32)
    w_left = np.asarray(w_left, dtype=np.float32)
    w_right = np.asarray(w_right, dtype=np.float32)
    b_l = float(np.asarray(b_left))
    b_r = float(np.asarray(b_right))

    # host-side parameter folding (weights only, no activations)
    v_left = (W_common.T @ w_left).astype(bf)
    v_right = (W_common.T @ w_right).astype(bf)
    b2 = b_l + b_r
    bias2 = np.array([b2, 0.2 * b2], dtype=np.float32)

    relb = relation.astype(bf)
    ctx_t = np.ascontiguousarray(context.T).astype(bf)  # [IN, N]

    in_maps = []
    for c in range(NCORES):
        sl = slice(c * R, (c + 1) * R)
        in_maps.append({
            "adjt": np.ascontiguousarray(adj_tensor[sl].T).astype(bf),
            "ctxt": ctx_t,
            "ctxot": np.ascontiguousarray(ctx_t[:, sl]),
            "rel_in": relb,
            "vl_in": v_left,
            "vr_in": v_right,
            "bias2": bias2,
        })
    return in_maps


# ------------------------------------------------------------------- entry
def kernel(relation, context, adj_tensor, W_common, w_left, b_left, w_right,
           b_right):
    from concourse.bass_utils import run_bass_kernel_spmd

    in_maps = prepare_in_maps(relation, context, adj_tensor, W_common,
                              w_left, b_left, w_right, b_right)
    nc = _get_program("main")
    last_err = None
    for _attempt in range(3):
        try:
            res = run_bass_kernel_spmd(nc, in_maps, list(range(NCORES)))
            outs = [res.results[c]["out"] for c in range(NCORES)]
            return np.concatenate(outs, axis=0).astype(np.float32)
        except Exception as e:  # transient device-unrecoverable seen on axon
            last_err = e
            import time as _time

            try:
                import jax

                jax.clear_caches()
            except Exception:
                pass
            _time.sleep(3.0)
    raise last_err
